# revision 25
# baseline (speedup 1.0000x reference)
"""Entmax (alpha=1.25) bisection kernel for Trainium2, 8 NeuronCores.

The reference solves  sum_j relu(x_j*0.25 - tau)^4 = 1  per row by 100
bisection steps and returns the normalized relu^4 probabilities.  With
tau' = 4*tau this is  F(tau') = sum_j relu(x_j - tau')^4 = 256.  Instead of
100 passes over the row, per 128-row tile we:
  1. compute 16-wide chunk maxima cmax[2000] while streaming the tile in,
  2. bisect G(t) = sum relu(cmax - t)^4 = 256 on a bf16 copy of cmax.
     G <= F pointwise, so the bracket's lower end minus a small margin for
     bf16 rounding is a guaranteed lower bound theta <= tau'*, and (top
     elements dominate the quartic sum) theta lands within ~3e-2 of tau'*,
  3. run one fused pass over the row for masked power sums at theta:
     A_m = sum relu(x - theta)^m, m = 1..4,
  4. solve the quartic expansion P(d) = A4 - 4 A3 d + 6 A2 d^2 - 4 A1 d^3
     = 256 per row with Newton (d = tau' - theta).  Elements inside
     (theta, tau') perturb P by <= count*d^4 ~ 1e-4 of 256, putting tau'
     within ~1e-7 of the reference's converged tau — at f32 resolution,
  5. one output pass p = relu(s*x - s*tau')^4 with s = P^{-1/4}: the
     1/sum(p) normalization is folded into the activation scale.

Work is spread over the engines: ACT runs the relus and two of the three
squares (with free accumulation), DVE runs the chunk-max reduce and the
two accumulating product ops, GPSIMD runs the two final non-accumulating
squares.  The row tile lives in 16 per-chunk SBUF tiles so the next row
tile's loads can start as soon as this tile's output DMA drains a chunk.
"""

import numpy as np

import concourse.bass as bass
import concourse.mybir as mybir
from concourse.tile import TileContext

P = 128                    # partitions
D = 32000                  # row length
ROWS_PER_CORE = 256        # 2048 / 8 cores
N_ROW_TILES = ROWS_PER_CORE // P   # 2
N_CORES = 8

CHUNK = 2000               # column chunk (load, moments, output)
N_CHUNKS = D // CHUNK      # 16
CW = 40                    # elements per chunk-max
CMAX_W = D // CW           # 2000
G_ITERS = 7
G_MARGIN = 0.008           # bf16 G-evaluation safety margin on theta
NEWTON_ITERS = 3

F32 = mybir.dt.float32
BF16 = mybir.dt.bfloat16
DM0 = float(np.float32(4.0 - 4.0 * (1.0 / D) ** 0.25))  # initial bracket width


def _row_tile(tc, pools, x_dram, out_dram, row0):
    nc = tc.nc
    xp, work, sink, small = pools
    Alu = mybir.AluOpType
    Act = mybir.ActivationFunctionType

    x_c = [
        xp.tile([P, CHUNK], F32, tag=f"x{c}", name=f"x{c}") for c in range(N_CHUNKS)
    ]
    cmax = sink.tile([P, CMAX_W], F32, tag="wm", name="cmax")

    # ---- load + chunk maxima (bf16 copy + row-max built incrementally) ----
    cmax_bf = small.tile([P, CMAX_W], BF16, tag="cmax_bf")
    rcol = small.tile([P, N_CHUNKS], F32, tag="rcol")
    for c in range(N_CHUNKS):
        sl = slice(c * (CHUNK // CW), (c + 1) * (CHUNK // CW))
        nc.sync.dma_start(
            out=x_c[c], in_=x_dram[row0 : row0 + P, c * CHUNK : (c + 1) * CHUNK]
        )
        nc.vector.reduce_max(
            out=cmax[:, sl],
            in_=x_c[c].rearrange("p (a b) -> p a b", b=CW),
            axis=mybir.AxisListType.X,
        )
        nc.vector.tensor_copy(cmax_bf[:, sl], cmax[:, sl])
        nc.vector.reduce_max(
            out=rcol[:, c : c + 1], in_=cmax[:, sl], axis=mybir.AxisListType.X
        )

    rmax = small.tile([P, 1], F32, tag="rmax")
    lo = small.tile([P, 1], F32, tag="lo")
    tm = small.tile([P, 1], F32, tag="tm")
    gv = small.tile([P, 1], F32, tag="gv")
    ind = small.tile([P, 1], F32, tag="ind")

    nc.vector.reduce_max(out=rmax, in_=rcol, axis=mybir.AxisListType.X)
    nc.vector.tensor_scalar(lo, rmax, 4.0, None, op0=Alu.subtract)

    for i in range(G_ITERS):
        dm_i = float(np.float32(DM0 * 0.5 ** (i + 1)))
        yg = work.tile([P, CMAX_W], BF16, tag="ym", name="yg")
        zg = work.tile([P, CMAX_W], BF16, tag="zm", name="zg")
        wg = sink.tile([P, CMAX_W], BF16, tag="wm", name="wg")
        nc.vector.tensor_scalar(tm, lo, dm_i, None, op0=Alu.add)
        nc.vector.tensor_scalar(yg, cmax_bf, tm, 0.0, op0=Alu.subtract, op1=Alu.max)
        nc.vector.tensor_mul(zg, yg, yg)
        nc.vector.scalar_tensor_tensor(
            out=wg, in0=zg, scalar=1.0, in1=zg, op0=Alu.mult, op1=Alu.mult,
            accum_out=gv,
        )
        nc.vector.tensor_scalar(ind, gv, 256.0, None, op0=Alu.is_ge)
        nc.vector.scalar_tensor_tensor(
            out=lo, in0=ind, scalar=dm_i, in1=lo, op0=Alu.mult, op1=Alu.add,
        )

    theta = small.tile([P, 1], F32, tag="theta")
    ntheta = small.tile([P, 1], F32, tag="ntheta")
    nc.vector.tensor_scalar(theta, lo, -G_MARGIN, None, op0=Alu.add)
    nc.vector.tensor_scalar(ntheta, theta, -1.0, None, op0=Alu.mult)

    # ---- fused moment pass: A1..A4 at theta ----
    a1p = small.tile([P, N_CHUNKS], F32, tag="a1p")
    a2p = small.tile([P, N_CHUNKS], F32, tag="a2p")
    a3p = small.tile([P, N_CHUNKS], F32, tag="a3p")
    a4p = small.tile([P, N_CHUNKS], F32, tag="a4p")
    for m in range(N_CHUNKS):
        ym = work.tile([P, CHUNK], F32, tag="ym")
        zm = work.tile([P, CHUNK], F32, tag="zm")
        wm = sink.tile([P, CHUNK], F32, tag="wm")
        vm = sink.tile([P, CHUNK], F32, tag="vm")
        nc.scalar.activation(
            ym, x_c[m], Act.Relu, bias=ntheta, scale=1.0,
            accum_out=a1p[:, m : m + 1],
        )
        nc.vector.scalar_tensor_tensor(
            out=zm, in0=ym, scalar=1.0, in1=ym, op0=Alu.mult, op1=Alu.mult,
            accum_out=a2p[:, m : m + 1],
        )
        nc.vector.scalar_tensor_tensor(
            out=wm, in0=zm, scalar=1.0, in1=ym, op0=Alu.mult, op1=Alu.mult,
            accum_out=a3p[:, m : m + 1],
        )
        nc.scalar.activation(vm, zm, Act.Square, accum_out=a4p[:, m : m + 1])

    a1 = small.tile([P, 1], F32, tag="a1")
    a2 = small.tile([P, 1], F32, tag="a2")
    a3 = small.tile([P, 1], F32, tag="a3")
    a4 = small.tile([P, 1], F32, tag="a4")
    for acc, prt in ((a1, a1p), (a2, a2p), (a3, a3p), (a4, a4p)):
        nc.vector.reduce_sum(out=acc, in_=prt, axis=mybir.AxisListType.X)

    # ---- Newton on P(d) = a4 + k1 d + k2 d^2 + k3 d^3 = 256 ----
    k1 = small.tile([P, 1], F32, tag="k1")
    k2 = small.tile([P, 1], F32, tag="k2")
    k3 = small.tile([P, 1], F32, tag="k3")
    q2 = small.tile([P, 1], F32, tag="q2")   # 2*k2
    q3 = small.tile([P, 1], F32, tag="q3")   # 3*k3
    dlt = small.tile([P, 1], F32, tag="dlt")
    pv = small.tile([P, 1], F32, tag="pv")
    ppv = small.tile([P, 1], F32, tag="ppv")
    stp = small.tile([P, 1], F32, tag="stp")

    a4m = small.tile([P, 1], F32, tag="a4m")
    nc.vector.tensor_scalar(a4m, a4, -256.0, None, op0=Alu.add)
    nc.vector.tensor_scalar(k1, a3, -4.0, None, op0=Alu.mult)
    nc.vector.tensor_scalar(k2, a2, 6.0, None, op0=Alu.mult)
    nc.vector.tensor_scalar(k3, a1, -4.0, None, op0=Alu.mult)
    nc.vector.tensor_scalar(q2, k2, 2.0, None, op0=Alu.mult)
    nc.vector.tensor_scalar(q3, k3, 3.0, None, op0=Alu.mult)
    nc.vector.memset(dlt, 0.0)

    def eval_P(dst, k0):
        nc.vector.tensor_mul(dst, k3, dlt)
        nc.vector.tensor_add(dst, dst, k2)
        nc.vector.tensor_mul(dst, dst, dlt)
        nc.vector.tensor_add(dst, dst, k1)
        nc.vector.tensor_mul(dst, dst, dlt)
        nc.vector.tensor_add(dst, dst, k0)

    for _ in range(NEWTON_ITERS):
        eval_P(pv, a4m)
        nc.vector.tensor_mul(ppv, q3, dlt)
        nc.vector.tensor_add(ppv, ppv, q2)
        nc.vector.tensor_mul(ppv, ppv, dlt)
        nc.vector.tensor_add(ppv, ppv, k1)
        nc.vector.reciprocal(ppv, ppv)
        nc.vector.tensor_mul(stp, pv, ppv)
        nc.vector.tensor_sub(dlt, dlt, stp)

    # tau' = theta + dlt ; s = P(dlt)^(-1/4) ; bias = -s*tau'
    taup = small.tile([P, 1], F32, tag="taup")
    sv = small.tile([P, 1], F32, tag="sv")
    nbias = small.tile([P, 1], F32, tag="nbias")
    eval_P(pv, a4)
    nc.vector.tensor_add(taup, theta, dlt)
    nc.vector.reciprocal(sv, pv)
    nc.scalar.activation(sv, sv, Act.Sqrt)
    nc.scalar.activation(sv, sv, Act.Sqrt)
    nc.vector.tensor_mul(nbias, sv, taup)
    nc.vector.tensor_scalar(nbias, nbias, -1.0, None, op0=Alu.mult)

    # ---- output pass: p = (relu(s*x - s*tau'))^4, written in place ----
    for m in range(N_CHUNKS):
        sl = slice(m * CHUNK, (m + 1) * CHUNK)
        yf = work.tile([P, CHUNK], F32, tag="ym")
        zf = work.tile([P, CHUNK], F32, tag="zm")
        nc.scalar.activation(yf, x_c[m], Act.Relu, bias=nbias, scale=sv)
        if m % 2 == 0:
            nc.gpsimd.tensor_mul(zf, yf, yf)
            nc.vector.scalar_tensor_tensor(
                out=x_c[m], in0=zf, scalar=1.0, in1=zf, op0=Alu.mult,
                op1=Alu.mult,
            )
        else:
            nc.vector.scalar_tensor_tensor(
                out=zf, in0=yf, scalar=1.0, in1=yf, op0=Alu.mult, op1=Alu.mult
            )
            nc.scalar.activation(x_c[m], zf, Act.Square)
        nc.sync.dma_start(out=out_dram[row0 : row0 + P, sl], in_=x_c[m])


def build_bass():
    from concourse import bacc

    nc = bacc.Bacc(None, target_bir_lowering=False)
    x_dram = nc.dram_tensor("x", [ROWS_PER_CORE, D], F32, kind="ExternalInput")
    out_dram = nc.dram_tensor("out", [ROWS_PER_CORE, D], F32, kind="ExternalOutput")
    with TileContext(nc) as tc:
        with (
            tc.tile_pool(name="xp", bufs=1) as xp,
            tc.tile_pool(name="work", bufs=4) as work,
            tc.tile_pool(name="sink", bufs=1) as sink,
            tc.tile_pool(name="small", bufs=1) as small,
        ):
            for t in range(N_ROW_TILES):
                _row_tile(tc, (xp, work, sink, small), x_dram, out_dram, t * P)
    nc.compile()
    return nc


_NC_CACHE = None


def kernel(input: np.ndarray) -> np.ndarray:
    global _NC_CACHE
    from concourse.bass_utils import run_bass_kernel_spmd

    x = np.ascontiguousarray(input, dtype=np.float32)
    assert x.shape == (ROWS_PER_CORE * N_CORES, D)

    if _NC_CACHE is None:
        _NC_CACHE = build_bass()
    nc = _NC_CACHE

    in_maps = [
        {"x": x[i * ROWS_PER_CORE : (i + 1) * ROWS_PER_CORE]} for i in range(N_CORES)
    ]
    res = run_bass_kernel_spmd(nc, in_maps, core_ids=list(range(N_CORES)))
    return np.concatenate([r["out"] for r in res.results], axis=0)


# revision 26
# speedup vs baseline: 1.0031x; 1.0031x over previous
"""Entmax (alpha=1.25) bisection kernel for Trainium2, 8 NeuronCores.

The reference solves  sum_j relu(x_j*0.25 - tau)^4 = 1  per row by 100
bisection steps and returns the normalized relu^4 probabilities.  With
tau' = 4*tau this is  F(tau') = sum_j relu(x_j - tau')^4 = 256.  Instead of
100 passes over the row, per 128-row tile we:
  1. compute 16-wide chunk maxima cmax[2000] while streaming the tile in,
  2. bisect G(t) = sum relu(cmax - t)^4 = 256 on a bf16 copy of cmax.
     G <= F pointwise, so the bracket's lower end minus a small margin for
     bf16 rounding is a guaranteed lower bound theta <= tau'*, and (top
     elements dominate the quartic sum) theta lands within ~3e-2 of tau'*,
  3. run one fused pass over the row for masked power sums at theta:
     A_m = sum relu(x - theta)^m, m = 1..4,
  4. solve the quartic expansion P(d) = A4 - 4 A3 d + 6 A2 d^2 - 4 A1 d^3
     = 256 per row with Newton (d = tau' - theta).  Elements inside
     (theta, tau') perturb P by <= count*d^4 ~ 1e-4 of 256, putting tau'
     within ~1e-7 of the reference's converged tau — at f32 resolution,
  5. one output pass p = relu(s*x - s*tau')^4 with s = P^{-1/4}: the
     1/sum(p) normalization is folded into the activation scale.

Work is spread over the engines: ACT runs the relus and two of the three
squares (with free accumulation), DVE runs the chunk-max reduce and the
two accumulating product ops, GPSIMD runs the two final non-accumulating
squares.  The row tile lives in 16 per-chunk SBUF tiles so the next row
tile's loads can start as soon as this tile's output DMA drains a chunk.
"""

import numpy as np

import concourse.bass as bass
import concourse.mybir as mybir
from concourse.tile import TileContext

P = 128                    # partitions
D = 32000                  # row length
ROWS_PER_CORE = 256        # 2048 / 8 cores
N_ROW_TILES = ROWS_PER_CORE // P   # 2
N_CORES = 8

CHUNK = 1600               # column chunk (load, moments, output)
N_CHUNKS = D // CHUNK      # 16
CW = 40                    # elements per chunk-max
CMAX_W = D // CW           # 2000
G_ITERS = 7
G_MARGIN = 0.008           # bf16 G-evaluation safety margin on theta
NEWTON_ITERS = 3

F32 = mybir.dt.float32
BF16 = mybir.dt.bfloat16
DM0 = float(np.float32(4.0 - 4.0 * (1.0 / D) ** 0.25))  # initial bracket width


def _row_tile(tc, pools, x_dram, out_dram, row0):
    nc = tc.nc
    xp, work, sink, small = pools
    Alu = mybir.AluOpType
    Act = mybir.ActivationFunctionType

    x_c = [
        xp.tile([P, CHUNK], F32, tag=f"x{c}", name=f"x{c}") for c in range(N_CHUNKS)
    ]
    cmax = sink.tile([P, CMAX_W], F32, tag="wm", name="cmax")

    # ---- load + chunk maxima (bf16 copy + row-max built incrementally) ----
    cmax_bf = small.tile([P, CMAX_W], BF16, tag="cmax_bf")
    rcol = small.tile([P, N_CHUNKS], F32, tag="rcol")
    for c in range(N_CHUNKS):
        sl = slice(c * (CHUNK // CW), (c + 1) * (CHUNK // CW))
        nc.sync.dma_start(
            out=x_c[c], in_=x_dram[row0 : row0 + P, c * CHUNK : (c + 1) * CHUNK]
        )
        nc.vector.reduce_max(
            out=cmax[:, sl],
            in_=x_c[c].rearrange("p (a b) -> p a b", b=CW),
            axis=mybir.AxisListType.X,
        )
        nc.vector.tensor_copy(cmax_bf[:, sl], cmax[:, sl])
        nc.vector.reduce_max(
            out=rcol[:, c : c + 1], in_=cmax[:, sl], axis=mybir.AxisListType.X
        )

    rmax = small.tile([P, 1], F32, tag="rmax")
    lo = small.tile([P, 1], F32, tag="lo")
    tm = small.tile([P, 1], F32, tag="tm")
    gv = small.tile([P, 1], F32, tag="gv")
    ind = small.tile([P, 1], F32, tag="ind")

    nc.vector.reduce_max(out=rmax, in_=rcol, axis=mybir.AxisListType.X)
    nc.vector.tensor_scalar(lo, rmax, 4.0, None, op0=Alu.subtract)

    for i in range(G_ITERS):
        dm_i = float(np.float32(DM0 * 0.5 ** (i + 1)))
        yg = work.tile([P, CMAX_W], BF16, tag="ym", name="yg")
        zg = work.tile([P, CMAX_W], BF16, tag="zm", name="zg")
        wg = sink.tile([P, CMAX_W], BF16, tag="wm", name="wg")
        nc.vector.tensor_scalar(tm, lo, dm_i, None, op0=Alu.add)
        nc.vector.tensor_scalar(yg, cmax_bf, tm, 0.0, op0=Alu.subtract, op1=Alu.max)
        nc.vector.tensor_mul(zg, yg, yg)
        nc.vector.scalar_tensor_tensor(
            out=wg, in0=zg, scalar=1.0, in1=zg, op0=Alu.mult, op1=Alu.mult,
            accum_out=gv,
        )
        nc.vector.tensor_scalar(ind, gv, 256.0, None, op0=Alu.is_ge)
        nc.vector.scalar_tensor_tensor(
            out=lo, in0=ind, scalar=dm_i, in1=lo, op0=Alu.mult, op1=Alu.add,
        )

    theta = small.tile([P, 1], F32, tag="theta")
    ntheta = small.tile([P, 1], F32, tag="ntheta")
    nc.vector.tensor_scalar(theta, lo, -G_MARGIN, None, op0=Alu.add)
    nc.vector.tensor_scalar(ntheta, theta, -1.0, None, op0=Alu.mult)

    # ---- fused moment pass: A1..A4 at theta ----
    a1p = small.tile([P, N_CHUNKS], F32, tag="a1p")
    a2p = small.tile([P, N_CHUNKS], F32, tag="a2p")
    a3p = small.tile([P, N_CHUNKS], F32, tag="a3p")
    a4p = small.tile([P, N_CHUNKS], F32, tag="a4p")
    for m in range(N_CHUNKS):
        ym = work.tile([P, CHUNK], F32, tag="ym")
        zm = work.tile([P, CHUNK], F32, tag="zm")
        wm = sink.tile([P, CHUNK], F32, tag="wm")
        vm = sink.tile([P, CHUNK], F32, tag="vm")
        nc.scalar.activation(
            ym, x_c[m], Act.Relu, bias=ntheta, scale=1.0,
            accum_out=a1p[:, m : m + 1],
        )
        nc.vector.scalar_tensor_tensor(
            out=zm, in0=ym, scalar=1.0, in1=ym, op0=Alu.mult, op1=Alu.mult,
            accum_out=a2p[:, m : m + 1],
        )
        nc.vector.scalar_tensor_tensor(
            out=wm, in0=zm, scalar=1.0, in1=ym, op0=Alu.mult, op1=Alu.mult,
            accum_out=a3p[:, m : m + 1],
        )
        nc.scalar.activation(vm, zm, Act.Square, accum_out=a4p[:, m : m + 1])

    a1 = small.tile([P, 1], F32, tag="a1")
    a2 = small.tile([P, 1], F32, tag="a2")
    a3 = small.tile([P, 1], F32, tag="a3")
    a4 = small.tile([P, 1], F32, tag="a4")
    for acc, prt in ((a1, a1p), (a2, a2p), (a3, a3p), (a4, a4p)):
        nc.vector.reduce_sum(out=acc, in_=prt, axis=mybir.AxisListType.X)

    # ---- Newton on P(d) = a4 + k1 d + k2 d^2 + k3 d^3 = 256 ----
    k1 = small.tile([P, 1], F32, tag="k1")
    k2 = small.tile([P, 1], F32, tag="k2")
    k3 = small.tile([P, 1], F32, tag="k3")
    q2 = small.tile([P, 1], F32, tag="q2")   # 2*k2
    q3 = small.tile([P, 1], F32, tag="q3")   # 3*k3
    dlt = small.tile([P, 1], F32, tag="dlt")
    pv = small.tile([P, 1], F32, tag="pv")
    ppv = small.tile([P, 1], F32, tag="ppv")
    stp = small.tile([P, 1], F32, tag="stp")

    a4m = small.tile([P, 1], F32, tag="a4m")
    nc.vector.tensor_scalar(a4m, a4, -256.0, None, op0=Alu.add)
    nc.vector.tensor_scalar(k1, a3, -4.0, None, op0=Alu.mult)
    nc.vector.tensor_scalar(k2, a2, 6.0, None, op0=Alu.mult)
    nc.vector.tensor_scalar(k3, a1, -4.0, None, op0=Alu.mult)
    nc.vector.tensor_scalar(q2, k2, 2.0, None, op0=Alu.mult)
    nc.vector.tensor_scalar(q3, k3, 3.0, None, op0=Alu.mult)
    nc.vector.memset(dlt, 0.0)

    def eval_P(dst, k0):
        nc.vector.tensor_mul(dst, k3, dlt)
        nc.vector.tensor_add(dst, dst, k2)
        nc.vector.tensor_mul(dst, dst, dlt)
        nc.vector.tensor_add(dst, dst, k1)
        nc.vector.tensor_mul(dst, dst, dlt)
        nc.vector.tensor_add(dst, dst, k0)

    for _ in range(NEWTON_ITERS):
        eval_P(pv, a4m)
        nc.vector.tensor_mul(ppv, q3, dlt)
        nc.vector.tensor_add(ppv, ppv, q2)
        nc.vector.tensor_mul(ppv, ppv, dlt)
        nc.vector.tensor_add(ppv, ppv, k1)
        nc.vector.reciprocal(ppv, ppv)
        nc.vector.tensor_mul(stp, pv, ppv)
        nc.vector.tensor_sub(dlt, dlt, stp)

    # tau' = theta + dlt ; s = P(dlt)^(-1/4) ; bias = -s*tau'
    taup = small.tile([P, 1], F32, tag="taup")
    sv = small.tile([P, 1], F32, tag="sv")
    nbias = small.tile([P, 1], F32, tag="nbias")
    eval_P(pv, a4)
    nc.vector.tensor_add(taup, theta, dlt)
    nc.vector.reciprocal(sv, pv)
    nc.scalar.activation(sv, sv, Act.Sqrt)
    nc.scalar.activation(sv, sv, Act.Sqrt)
    nc.vector.tensor_mul(nbias, sv, taup)
    nc.vector.tensor_scalar(nbias, nbias, -1.0, None, op0=Alu.mult)

    # ---- output pass: p = (relu(s*x - s*tau'))^4, written in place ----
    for m in range(N_CHUNKS):
        sl = slice(m * CHUNK, (m + 1) * CHUNK)
        yf = work.tile([P, CHUNK], F32, tag="ym")
        zf = work.tile([P, CHUNK], F32, tag="zm")
        nc.scalar.activation(yf, x_c[m], Act.Relu, bias=nbias, scale=sv)
        if m % 2 == 0:
            nc.gpsimd.tensor_mul(zf, yf, yf)
            nc.vector.scalar_tensor_tensor(
                out=x_c[m], in0=zf, scalar=1.0, in1=zf, op0=Alu.mult,
                op1=Alu.mult,
            )
        else:
            nc.vector.scalar_tensor_tensor(
                out=zf, in0=yf, scalar=1.0, in1=yf, op0=Alu.mult, op1=Alu.mult
            )
            nc.scalar.activation(x_c[m], zf, Act.Square)
        nc.sync.dma_start(out=out_dram[row0 : row0 + P, sl], in_=x_c[m])


def build_bass():
    from concourse import bacc

    nc = bacc.Bacc(None, target_bir_lowering=False)
    x_dram = nc.dram_tensor("x", [ROWS_PER_CORE, D], F32, kind="ExternalInput")
    out_dram = nc.dram_tensor("out", [ROWS_PER_CORE, D], F32, kind="ExternalOutput")
    with TileContext(nc) as tc:
        with (
            tc.tile_pool(name="xp", bufs=1) as xp,
            tc.tile_pool(name="work", bufs=5) as work,
            tc.tile_pool(name="sink", bufs=1) as sink,
            tc.tile_pool(name="small", bufs=1) as small,
        ):
            for t in range(N_ROW_TILES):
                _row_tile(tc, (xp, work, sink, small), x_dram, out_dram, t * P)
    nc.compile()
    return nc


_NC_CACHE = None


def kernel(input: np.ndarray) -> np.ndarray:
    global _NC_CACHE
    from concourse.bass_utils import run_bass_kernel_spmd

    x = np.ascontiguousarray(input, dtype=np.float32)
    assert x.shape == (ROWS_PER_CORE * N_CORES, D)

    if _NC_CACHE is None:
        _NC_CACHE = build_bass()
    nc = _NC_CACHE

    in_maps = [
        {"x": x[i * ROWS_PER_CORE : (i + 1) * ROWS_PER_CORE]} for i in range(N_CORES)
    ]
    res = run_bass_kernel_spmd(nc, in_maps, core_ids=list(range(N_CORES)))
    return np.concatenate([r["out"] for r in res.results], axis=0)
